# revision 31
# baseline (speedup 1.0000x reference)
"""DESOM (deep-embedded SOM) Trainium2 kernel.

Strategy
--------
Data-parallel over the batch: each of the 8 NeuronCores gets B/8 = 2048 rows.
On-device everything runs in bf16 (fp32 PSUM accumulation, fp32 outputs);
host-side we cast weights/x to bf16 and pre-build two augmented operands:

  * protoT_aug [258, 1024] = vstack(-2 * P.T, ones, p2)   (SOM distance GEMM)
  * dW0aug    [2001, 784]  = vstack(dW0, db0)             (decoder output bias)

Activations live in SBUF feature-major ([features(part), batch(free)]) so every
Dense layer is a plain lhsT=W[k,:][:,m] / rhs=h[k] matmul chain with PSUM
accumulation over k, evicted through ScalarE as relu(psum)+bias.  The two
batch-major outputs (decoded, d) swap operand roles (stationary = activation
column block) so they come out row-major and DMA straight to DRAM.

The batch is processed in 2 passes of 1024 columns per core; weights stream
from HBM each pass (~14 MB bf16 * 2 per core, well under the PE time).

This walrus build only accepts ONE semaphore wait per instruction, while Tile
emits several; `_split_multiwaits` hoists extra waits onto same-engine NoOps.
"""

import os
import sys

for _p in ("/opt/trn_rl_repo",):
    if os.path.isdir(_p) and _p not in sys.path:
        try:
            import concourse  # noqa: F401
        except Exception:
            sys.path.insert(0, _p)
    break

import numpy as np
import ml_dtypes

import concourse.bass as bass
import concourse.tile as tile
import bass_rust as _br
from concourse import mybir
from concourse.bass_utils import run_bass_kernel_spmd

BF16 = mybir.dt.bfloat16
F32 = mybir.dt.float32
AF = mybir.ActivationFunctionType

N_CORES = 8
B_TOTAL = 16384
B_SHARD = B_TOTAL // N_CORES          # 2048
BT = 1024                             # batch columns per pass
PASSES = B_SHARD // BT                # 2
DIMS = [784, 2000, 2000, 500, 256]    # encoder dims + latent
P_PROTO = 1024
LATENT = 256
MG = 4                                # m-tiles per PSUM group (4 x 2-bank tiles)


def _chunks(total, step=128):
    out = []
    k0 = 0
    while k0 < total:
        out.append((k0, min(step, total - k0)))
        k0 += step
    return out


def _split_multiwaits(nc):
    """Hoist all-but-one semaphore waits onto same-engine NoOps (this
    walrus rejects >1 sync wait per instruction)."""
    n = 0
    for f in nc.m.functions:
        for blk in f.blocks:
            insts = list(blk.instructions)
            out = []
            changed = False
            for inst in insts:
                si = inst.sync_info
                if si is not None and len(si.on_wait) > 1:
                    waits = list(si.on_wait)
                    for j, w in enumerate(waits[:-1]):
                        nop = mybir.InstNoOp(name=f"{inst.name}-hw{j}", ins=[], outs=[])
                        nop.engine = inst.engine
                        nop.sync_info = _br.SyncInfo(on_wait=[w], on_update=[])
                        out.append(nop)
                        n += 1
                    inst.sync_info = _br.SyncInfo(
                        on_wait=[waits[-1]], on_update=list(si.on_update)
                    )
                    changed = True
                out.append(inst)
            if changed:
                blk.instructions = out
    return n


def _dedupe_ldweights(nc):
    """Remove back-to-back InstLdweights with identical weight APs (Tile
    emits one per matmul; consecutive matmuls often reuse the stationary
    operand).  Only drops an LDW carrying no semaphore waits/updates, and
    resets tracking at any other PE instruction."""
    removed = 0
    for f in nc.m.functions:
        for blk in f.blocks:
            insts = list(blk.instructions)
            out = []
            last_key = None
            changed = False
            for inst in insts:
                if str(inst.engine) != "EngineType.PE":
                    out.append(inst)
                    continue
                tn = type(inst).__name__
                if tn == "InstLdweights":
                    key = (str(inst.ins[0]), str(getattr(inst, "is_transpose", None)),
                           str(getattr(inst, "perf_mode", None)),
                           str(getattr(inst, "tile_position", None)))
                    si = inst.sync_info
                    bare = si is None or (not si.on_wait and not si.on_update)
                    if key == last_key and bare:
                        removed += 1
                        changed = True
                        continue
                    last_key = key
                elif tn != "InstMatmult":
                    last_key = None
                out.append(inst)
            if changed:
                blk.instructions = out
    return removed


def _build_program(has_bias, repeat=1):
    """Build the SPMD Bass program (one core's view: a [B_SHARD] slice).

    repeat>1 wraps the whole body in a hardware For_i loop — a timing-only
    variant used to amortize the remote-dispatch overhead when measuring."""
    nc = bass.Bass("TRN2", target_bir_lowering=False, debug=False)

    dt_in = {}

    def din(name, shape, dt=BF16):
        dt_in[name] = nc.dram_tensor(name, list(shape), dt, kind="ExternalInput").ap()
        return dt_in[name]

    XT = din("xt", (DIMS[0], B_SHARD))
    eW = [din(f"eW{i}", (DIMS[i], DIMS[i + 1])) for i in range(4)]
    dW = [din(f"dW{j}", (DIMS[j + 1], DIMS[j])) for j in (3, 2, 1)]  # dW3,dW2,dW1
    DW0AUG = din("dW0aug", (DIMS[1] + 1, DIMS[0]))
    PROTO = din("protoT_aug", (LATENT + 2, P_PROTO))
    ONESR = din("ones_r", (1, B_SHARD))
    ebias = {}
    for nm, m in [("eb0", 2000), ("eb1", 2000), ("eb2", 500), ("eb3", 256),
                  ("db3", 500), ("db2", 2000), ("db1", 2000)]:
        if has_bias:
            ebias[nm] = din(nm, (m,), F32)
        else:
            ebias[nm] = None

    DEC = nc.dram_tensor("decoded", [B_SHARD, DIMS[0]], F32, kind="ExternalOutput").ap()
    DOUT = nc.dram_tensor("dist", [B_SHARD, P_PROTO], F32, kind="ExternalOutput").ap()

    with tile.TileContext(nc) as tc:
        with tc.tile_pool(name="wpool", bufs=26) as wpool, \
             tc.tile_pool(name="apool", bufs=36) as apool, \
             tc.tile_pool(name="opool", bufs=4) as opool, \
             tc.tile_pool(name="cpool", bufs=1) as cpool, \
             tc.tile_pool(name="bpool", bufs=8) as bpool, \
             tc.tile_pool(name="proto_pool", bufs=4) as proto_pool, \
             tc.tile_pool(name="pspool", bufs=4, space="PSUM") as pspool:

            # constants / prototypes (once)
            ones_col = cpool.tile([128, 1], BF16, name="ones", tag="c")
            nc.vector.memset(ones_col[:, :], 1.0)
            # proto chunks: [128, 128, 2] - the (ones; p2) tail is one tile
            ZCH = [(0, 128), (128, 128), (256, 2)]
            proto_tiles = []
            for (k0, kk) in ZCH:
                t = proto_pool.tile([kk, P_PROTO], BF16, name="proto_t", tag="proto")
                nc.sync.dma_start(t[:, :], PROTO[k0:k0 + kk, :])
                proto_tiles.append(t)

            def feat_layer(h_in, W, K, M, relu, bias_ap, extra_row=False):
                """Feature-major dense layer: returns list of out tiles
                ([mm, BT] bf16 per 128-chunk of M).  h_in: k-chunk tiles."""
                kch = _chunks(K)
                mch = _chunks(M)
                assert len(h_in) == len(kch)
                wt = []
                for (k0, kk) in kch:
                    t = wpool.tile([kk, M], BF16, name="w_t", tag="w")
                    nc.sync.dma_start(t[:, :], W[k0:k0 + kk, :])
                    wt.append(t)
                bias_tiles = {}
                if bias_ap is not None:
                    for mi, (m0, mm) in enumerate(mch):
                        bt_ = bpool.tile([mm, 1], F32, name="b_t", tag="b")
                        nc.sync.dma_start(bt_[:, :], bias_ap[m0:m0 + mm].unsqueeze(1))
                        bias_tiles[mi] = bt_
                out = []
                for g0 in range(0, len(mch), MG):
                    grp = list(range(g0, min(g0 + MG, len(mch))))
                    ps = {}
                    for mi in grp:
                        m0, mm = mch[mi]
                        ps[mi] = pspool.tile([mm, BT], F32, name="ps_t", tag="ps")
                    ks = list(range(len(kch)))
                    if (g0 // MG) % 2 == 1:
                        ks = ks[::-1]
                    for pos, ki in enumerate(ks):
                        first, last = pos == 0, pos == len(ks) - 1
                        kk = kch[ki][1]
                        for mi in grp:
                            m0, mm = mch[mi]
                            for nb in range(0, BT, 512):
                                nc.tensor.matmul(
                                    ps[mi][:, nb:nb + 512],
                                    wt[ki][:kk, m0:m0 + mm],
                                    h_in[ki][:kk, nb:nb + 512],
                                    start=first, stop=last,
                                )
                    for mi in grp:
                        m0, mm = mch[mi]
                        amm = mm + 1 if (extra_row and mi == len(mch) - 1) else mm
                        ot = apool.tile([amm, BT], BF16, name="act_t", tag="a")
                        bias = bias_tiles.get(mi, 0.0)
                        bias = bias[:, :] if not isinstance(bias, float) else bias
                        nc.scalar.activation(
                            ot[:mm, :], ps[mi][:, :],
                            AF.Relu if relu else AF.Copy, bias=bias,
                        )
                        out.append(ot)
                return out

            import contextlib
            rep_ctx = tc.For_i(0, repeat, 1) if repeat > 1 else contextlib.nullcontext()
            with rep_ctx:
              for p in range(PASSES):
                b0 = p * BT
                # ---- load x.T slice (feature-major input)
                h = []
                for (k0, kk) in _chunks(DIMS[0]):
                    t = apool.tile([kk, BT], BF16, name="xt_t", tag="a")
                    nc.sync.dma_start(t[:, :], XT[k0:k0 + kk, b0:b0 + BT])
                    h.append(t)

                # ---- encoder
                h = feat_layer(h, eW[0], DIMS[0], DIMS[1], True, ebias["eb0"])
                h = feat_layer(h, eW[1], DIMS[1], DIMS[2], True, ebias["eb1"])
                h = feat_layer(h, eW[2], DIMS[2], DIMS[3], True, ebias["eb2"])
                z = feat_layer(h, eW[3], DIMS[3], LATENT, False, ebias["eb3"])

                # ---- SOM prep: zaug = [z0, z1, z2row, onesrow]
                zsq = []
                for zt in z:
                    s = apool.tile([128, BT], BF16, name="zsq_t", tag="a")
                    nc.vector.tensor_mul(s[:, :], zt[:, :], zt[:, :])
                    zsq.append(s)
                ztail = apool.tile([2, BT], BF16, name="ztail", tag="a")
                psz = pspool.tile([1, BT], F32, name="psz", tag="ps")
                for ki in range(2):
                    for nb in range(0, BT, 512):
                        nc.tensor.matmul(psz[:, nb:nb + 512], ones_col[:, :],
                                         zsq[ki][:, nb:nb + 512],
                                         start=(ki == 0), stop=(ki == 1))
                nc.scalar.activation(ztail[0:1, :], psz[:, :], AF.Copy)
                nc.sync.dma_start(ztail[1:2, :], ONESR[:, b0:b0 + BT])
                zaug = z + [ztail]

                # ---- SOM distances (batch-major out)
                zch = ZCH
                for bt_i in range(BT // 128):
                    bsl = slice(bt_i * 128, (bt_i + 1) * 128)
                    dtile = opool.tile([128, P_PROTO], F32, name="dtile", tag="o")
                    # ki outer / half inner so both 512-halves share one
                    # stationary load (deduped by _dedupe_ldweights)
                    psd = [pspool.tile([128, 512], F32, name="psd", tag="ps")
                           for _ in range(2)]
                    for ki in range(len(zch)):
                        for half in range(2):
                            nsl = slice(half * 512, (half + 1) * 512)
                            nc.tensor.matmul(
                                psd[half][:, :], zaug[ki][:, bsl],
                                proto_tiles[ki][:, nsl],
                                start=(ki == 0), stop=(ki == len(zch) - 1),
                            )
                    for half in range(2):
                        nsl = slice(half * 512, (half + 1) * 512)
                        nc.scalar.activation(dtile[:, nsl], psd[half][:, :], AF.Relu)
                    nc.sync.dma_start(DOUT[b0 + bt_i * 128: b0 + (bt_i + 1) * 128, :],
                                      dtile[:, :])

                # ---- decoder
                h = feat_layer(z, dW[0], LATENT, DIMS[3], True, ebias["db3"])
                h = feat_layer(h, dW[1], DIMS[3], DIMS[2], True, ebias["db2"])
                h = feat_layer(h, dW[2], DIMS[2], DIMS[1], True, ebias["db1"],
                               extra_row=True)
                # fused output-bias row (matches dW0aug row 2000)
                nc.sync.dma_start(h[-1][80:81, :], ONESR[:, b0:b0 + BT])

                # ---- decoder output layer (batch-major out, bias folded)
                kch = _chunks(DIMS[1])
                kch = kch[:-1] + [(kch[-1][0], kch[-1][1] + 1)]  # 80 -> 81 rows
                w0t = []
                for (k0, kk) in kch:
                    t = wpool.tile([kk, DIMS[0]], BF16, name="w0_t", tag="w")
                    nc.sync.dma_start(t[:, :], DW0AUG[k0:k0 + kk, :])
                    w0t.append(t)
                NSPLIT = ((0, 512), (512, DIMS[0] - 512))
                for bt_i in range(BT // 128):
                    bsl = slice(bt_i * 128, (bt_i + 1) * 128)
                    dec = opool.tile([128, DIMS[0]], F32, name="dec_t", tag="o")
                    psd2 = [pspool.tile([128, nn], F32, name="psd2", tag="ps")
                            for (n0, nn) in NSPLIT]
                    for ki in range(len(kch)):
                        for hi, (n0, nn) in enumerate(NSPLIT):
                            nc.tensor.matmul(
                                psd2[hi][:, :], h[ki][:, bsl], w0t[ki][:, n0:n0 + nn],
                                start=(ki == 0), stop=(ki == len(kch) - 1),
                            )
                    for hi, (n0, nn) in enumerate(NSPLIT):
                        nc.scalar.activation(dec[:, n0:n0 + nn], psd2[hi][:, :],
                                             AF.Copy)
                    nc.sync.dma_start(DEC[b0 + bt_i * 128: b0 + (bt_i + 1) * 128, :],
                                      dec[:, :])

    if os.environ.get("DESOM_NO_DEDUP", "0") != "1":
        _dedupe_ldweights(nc)
    _split_multiwaits(nc)
    return nc


_PROGRAM_CACHE = {}


def _get_program(has_bias, repeat=1):
    key = (has_bias, repeat, os.environ.get("DESOM_NO_DEDUP", "0"))
    if key not in _PROGRAM_CACHE:
        _PROGRAM_CACHE[key] = _build_program(has_bias, repeat)
    return _PROGRAM_CACHE[key]


def _prepare_in_maps(inputs):
    bf = ml_dtypes.bfloat16
    f32 = np.float32

    def as_np(v, dt=f32):
        return np.asarray(v, dtype=dt)

    x = as_np(inputs["x"])
    P = as_np(inputs["prototypes"])
    dW0 = as_np(inputs["dW0"])
    db0 = as_np(inputs["db0"])

    xt = np.ascontiguousarray(x.astype(bf).T)           # [784, B_TOTAL]
    p2 = (P.astype(np.float64) ** 2).sum(1).astype(f32)
    proto_aug = np.concatenate(
        [(-2.0 * P.T), np.ones((1, P_PROTO), f32), p2[None, :]], axis=0
    ).astype(bf)                                        # [258, 1024]
    dW0aug = np.concatenate([dW0, db0[None, :]], axis=0).astype(bf)  # [2001, 784]

    weights = {f"eW{i}": as_np(inputs[f"eW{i}"]).astype(bf) for i in range(4)}
    weights.update({f"dW{j}": as_np(inputs[f"dW{j}"]).astype(bf) for j in (3, 2, 1)})

    biases = {nm: as_np(inputs[nm]) for nm in
              ("eb0", "eb1", "eb2", "eb3", "db3", "db2", "db1")}
    has_bias = any(np.any(b != 0) for b in biases.values())

    in_maps = []
    for c in range(N_CORES):
        m = {
            "xt": np.ascontiguousarray(xt[:, c * B_SHARD:(c + 1) * B_SHARD]),
            "dW0aug": dW0aug,
            "protoT_aug": proto_aug,
            "ones_r": np.ones((1, B_SHARD), bf),
        }
        m.update(weights)
        if has_bias:
            m.update(biases)
        in_maps.append(m)
    return in_maps, has_bias


def _run(inputs, trace=False):
    in_maps, has_bias = _prepare_in_maps(inputs)
    nc = _get_program(has_bias)
    res = run_bass_kernel_spmd(nc, in_maps, core_ids=list(range(N_CORES)),
                               trace=trace)
    decoded = np.concatenate([res.results[c]["decoded"] for c in range(N_CORES)], axis=0)
    dist = np.concatenate([res.results[c]["dist"] for c in range(N_CORES)], axis=0)
    return (decoded, dist), res


def kernel(**inputs):
    import time as _time
    last = None
    for attempt in range(3):
        try:
            (decoded, dist), _ = _run(inputs, trace=False)
            return decoded, dist
        except Exception as e:  # transient NRT/dispatch failure: retry
            last = e
            _time.sleep(5 * (attempt + 1))
    raise last


# ---------------------------------------------------------------------------
# Timing path: cached jit executable + device-resident inputs, so repeated
# calls measure dispatch+execute without the ~250 MB axon input transfer.
# ---------------------------------------------------------------------------
_EXEC_CACHE = {}


def _get_executable(nc, chain=1):
    if (id(nc), chain) in _EXEC_CACHE:
        return _EXEC_CACHE[(id(nc), chain)]
    import jax
    from concourse.bass2jax import (
        _bass_exec_p, install_neuronx_cc_hook, partition_id_tensor)
    from jax.experimental.shard_map import shard_map
    from jax.sharding import Mesh, PartitionSpec

    install_neuronx_cc_hook()
    part_name = nc.partition_id_tensor.name if nc.partition_id_tensor else None
    in_names, out_names, out_avals = [], [], []
    for alloc in nc.m.functions[0].allocations:
        if not isinstance(alloc, mybir.MemoryLocationSet):
            continue
        name = alloc.memorylocations[0].name
        if alloc.kind == "ExternalInput":
            if name != part_name:
                in_names.append(name)
        elif alloc.kind == "ExternalOutput":
            out_names.append(name)
            out_avals.append(jax.core.ShapedArray(
                tuple(alloc.tensor_shape), mybir.dt.np(alloc.dtype)))
    n_params = len(in_names)
    all_names = list(in_names) + list(out_names)
    if part_name is not None:
        all_names.append(part_name)
    all_names = tuple(all_names)

    n_outs = len(out_names)

    def _body(*args):
        ins = list(args[:n_params])
        state = list(args[n_params:n_params + n_outs])
        pid = [partition_id_tensor()] if part_name is not None else []
        for _ in range(chain):
            state = list(_bass_exec_p.bind(
                *ins, *state, *pid, out_avals=tuple(out_avals),
                in_names=all_names, out_names=tuple(out_names),
                lowering_input_output_aliases=(),
                sim_require_finite=True, sim_require_nnan=True, nc=nc))
        return tuple(state)

    devices = jax.devices()[:N_CORES]
    mesh = Mesh(np.asarray(devices), ("core",))
    nio = n_params + len(out_names)
    fn = jax.jit(
        shard_map(_body, mesh=mesh,
                  in_specs=(PartitionSpec("core"),) * nio,
                  out_specs=(PartitionSpec("core"),) * len(out_names),
                  check_rep=False),
        keep_unused=True,
    )
    _EXEC_CACHE[(id(nc), chain)] = (fn, in_names, out_names, out_avals, mesh)
    return _EXEC_CACHE[(id(nc), chain)]


def _run_on_device(nc, in_maps, iters):
    """Dispatch nc via a cached jit executable with device-resident inputs;
    returns (per-core results, list of wall times)."""
    import time as _time
    import jax
    from jax.sharding import NamedSharding, PartitionSpec

    fn, in_names, out_names, out_avals, mesh = _get_executable(nc)
    sh = NamedSharding(mesh, PartitionSpec("core"))
    concat_in = [np.concatenate([m[name] for m in in_maps], axis=0)
                 for name in in_names]
    zeros = [np.zeros((N_CORES * a.shape[0], *a.shape[1:]), a.dtype)
             for a in out_avals]
    dev_args = [jax.device_put(a, sh) for a in concat_in + zeros]
    outs = fn(*dev_args)
    jax.block_until_ready(outs)
    times = []
    for _ in range(iters):
        t0 = _time.perf_counter()
        jax.block_until_ready(fn(*dev_args))
        times.append(_time.perf_counter() - t0)
    res = {}
    for i, name in enumerate(out_names):
        arr = np.asarray(outs[i]).reshape(N_CORES, *out_avals[i].shape)
        res[name] = np.concatenate(list(arr), axis=0)
    return res, times


def _timed_run(inputs, iters=6, repeat=32):
    """Returns outputs plus an estimated per-execution time.  The kernel body
    is repeated `repeat` times inside one NEFF via a hardware loop so the
    ~50-110 ms remote-dispatch overhead amortizes away:
        per_exec = (T(repeat) - T(1)) / (repeat - 1)."""
    in_maps, has_bias = _prepare_in_maps(inputs)
    res1, t1s = _run_on_device(_get_program(has_bias, 1), in_maps, iters)
    _, tNs = _run_on_device(_get_program(has_bias, repeat), in_maps, iters)
    per_exec = (min(tNs) - min(t1s)) / (repeat - 1)
    return (res1["decoded"], res1["dist"]), {
        "per_exec_s": per_exec, "t1_ms": [t * 1e3 for t in t1s],
        "tN_ms": [t * 1e3 for t in tNs], "chain": repeat,
    }


# revision 33
# speedup vs baseline: 1.1511x; 1.1511x over previous
"""DESOM (deep-embedded SOM) Trainium2 kernel.

Strategy
--------
Data-parallel over the batch: each of the 8 NeuronCores gets B/8 = 2048 rows.
On-device everything runs in bf16 (fp32 PSUM accumulation, fp32 outputs);
host-side we cast weights/x to bf16 and pre-build two augmented operands:

  * protoT_aug [258, 1024] = vstack(-2 * P.T, ones, p2)   (SOM distance GEMM)
  * dW0aug    [2001, 784]  = vstack(dW0, db0)             (decoder output bias)

Activations live in SBUF feature-major ([features(part), batch(free)]) so every
Dense layer is a plain lhsT=W[k,:][:,m] / rhs=h[k] matmul chain with PSUM
accumulation over k, evicted through ScalarE as relu(psum)+bias.  The two
batch-major outputs (decoded, d) swap operand roles (stationary = activation
column block) so they come out row-major and DMA straight to DRAM.

The batch is processed in 2 passes of 1024 columns per core; weights stream
from HBM each pass (~14 MB bf16 * 2 per core, well under the PE time).

This walrus build only accepts ONE semaphore wait per instruction, while Tile
emits several; `_split_multiwaits` hoists extra waits onto same-engine NoOps.
"""

import os
import sys

for _p in ("/opt/trn_rl_repo",):
    if os.path.isdir(_p) and _p not in sys.path:
        try:
            import concourse  # noqa: F401
        except Exception:
            sys.path.insert(0, _p)
    break

import numpy as np
import ml_dtypes

import concourse.bass as bass
import concourse.tile as tile
import bass_rust as _br
from concourse import mybir
from concourse.bass_utils import run_bass_kernel_spmd

BF16 = mybir.dt.bfloat16
F32 = mybir.dt.float32
AF = mybir.ActivationFunctionType

N_CORES = 8
B_TOTAL = 16384
B_SHARD = B_TOTAL // N_CORES          # 2048
BT = 1024                             # batch columns per pass
PASSES = B_SHARD // BT                # 2
DIMS = [784, 2000, 2000, 500, 256]    # encoder dims + latent
P_PROTO = 1024
LATENT = 256
MG = 3                                # m-tiles per PSUM group (one spare 2-bank slot for overlap)


def _chunks(total, step=128):
    out = []
    k0 = 0
    while k0 < total:
        out.append((k0, min(step, total - k0)))
        k0 += step
    return out


def _split_multiwaits(nc):
    """Hoist all-but-one semaphore waits onto same-engine NoOps (this
    walrus rejects >1 sync wait per instruction)."""
    n = 0
    for f in nc.m.functions:
        for blk in f.blocks:
            insts = list(blk.instructions)
            out = []
            changed = False
            for inst in insts:
                si = inst.sync_info
                if si is not None and len(si.on_wait) > 1:
                    waits = list(si.on_wait)
                    for j, w in enumerate(waits[:-1]):
                        nop = mybir.InstNoOp(name=f"{inst.name}-hw{j}", ins=[], outs=[])
                        nop.engine = inst.engine
                        nop.sync_info = _br.SyncInfo(on_wait=[w], on_update=[])
                        out.append(nop)
                        n += 1
                    inst.sync_info = _br.SyncInfo(
                        on_wait=[waits[-1]], on_update=list(si.on_update)
                    )
                    changed = True
                out.append(inst)
            if changed:
                blk.instructions = out
    return n


def _dedupe_ldweights(nc):
    """Remove back-to-back InstLdweights with identical weight APs (Tile
    emits one per matmul; consecutive matmuls often reuse the stationary
    operand).  Only drops an LDW carrying no semaphore waits/updates, and
    resets tracking at any other PE instruction."""
    removed = 0
    for f in nc.m.functions:
        for blk in f.blocks:
            insts = list(blk.instructions)
            out = []
            last_key = None
            changed = False
            for inst in insts:
                if str(inst.engine) != "EngineType.PE":
                    out.append(inst)
                    continue
                tn = type(inst).__name__
                if tn == "InstLdweights":
                    key = (str(inst.ins[0]), str(getattr(inst, "is_transpose", None)),
                           str(getattr(inst, "perf_mode", None)),
                           str(getattr(inst, "tile_position", None)))
                    si = inst.sync_info
                    bare = si is None or (not si.on_wait and not si.on_update)
                    if key == last_key and bare:
                        removed += 1
                        changed = True
                        continue
                    last_key = key
                elif tn != "InstMatmult":
                    last_key = None
                out.append(inst)
            if changed:
                blk.instructions = out
    return removed


def _build_program(has_bias, repeat=1):
    """Build the SPMD Bass program (one core's view: a [B_SHARD] slice).

    repeat>1 wraps the whole body in a hardware For_i loop — a timing-only
    variant used to amortize the remote-dispatch overhead when measuring."""
    nc = bass.Bass("TRN2", target_bir_lowering=False, debug=False)

    dt_in = {}

    def din(name, shape, dt=BF16):
        dt_in[name] = nc.dram_tensor(name, list(shape), dt, kind="ExternalInput").ap()
        return dt_in[name]

    XT = din("xt", (DIMS[0], B_SHARD))
    eW = [din(f"eW{i}", (DIMS[i], DIMS[i + 1])) for i in range(4)]
    dW = [din(f"dW{j}", (DIMS[j + 1], DIMS[j])) for j in (3, 2, 1)]  # dW3,dW2,dW1
    DW0AUG = din("dW0aug", (DIMS[1] + 1, DIMS[0]))
    PROTO = din("protoT_aug", (LATENT + 2, P_PROTO))
    ONESR = din("ones_r", (1, B_SHARD))
    ebias = {}
    for nm, m in [("eb0", 2000), ("eb1", 2000), ("eb2", 500), ("eb3", 256),
                  ("db3", 500), ("db2", 2000), ("db1", 2000)]:
        if has_bias:
            ebias[nm] = din(nm, (m,), F32)
        else:
            ebias[nm] = None

    DEC = nc.dram_tensor("decoded", [B_SHARD, DIMS[0]], F32, kind="ExternalOutput").ap()
    DOUT = nc.dram_tensor("dist", [B_SHARD, P_PROTO], F32, kind="ExternalOutput").ap()

    with tile.TileContext(nc) as tc:
        with tc.tile_pool(name="wpool", bufs=24) as wpool, \
             tc.tile_pool(name="apool", bufs=34) as apool, \
             tc.tile_pool(name="opool", bufs=4) as opool, \
             tc.tile_pool(name="cpool", bufs=1) as cpool, \
             tc.tile_pool(name="bpool", bufs=8) as bpool, \
             tc.tile_pool(name="proto_pool", bufs=4) as proto_pool, \
             tc.tile_pool(name="pspool", bufs=4, space="PSUM") as pspool:

            # constants / prototypes (once)
            ones_col = cpool.tile([128, 1], BF16, name="ones", tag="c")
            nc.vector.memset(ones_col[:, :], 1.0)
            # proto chunks: [128, 128, 2] - the (ones; p2) tail is one tile
            ZCH = [(0, 128), (128, 128), (256, 2)]
            proto_tiles = []
            for (k0, kk) in ZCH:
                t = proto_pool.tile([kk, P_PROTO], BF16, name="proto_t", tag="proto")
                nc.sync.dma_start(t[:, :], PROTO[k0:k0 + kk, :])
                proto_tiles.append(t)

            def feat_layer(h_in, W, K, M, relu, bias_ap, extra_row=False):
                """Feature-major dense layer: returns list of out tiles
                ([mm, BT] bf16 per 128-chunk of M).  h_in: k-chunk tiles."""
                kch = _chunks(K)
                mch = _chunks(M)
                assert len(h_in) == len(kch)
                wt = []
                for (k0, kk) in kch:
                    t = wpool.tile([kk, M], BF16, name="w_t", tag="w")
                    nc.sync.dma_start(t[:, :], W[k0:k0 + kk, :])
                    wt.append(t)
                bias_tiles = {}
                if bias_ap is not None:
                    for mi, (m0, mm) in enumerate(mch):
                        bt_ = bpool.tile([mm, 1], F32, name="b_t", tag="b")
                        nc.sync.dma_start(bt_[:, :], bias_ap[m0:m0 + mm].unsqueeze(1))
                        bias_tiles[mi] = bt_
                out = []
                for g0 in range(0, len(mch), MG):
                    grp = list(range(g0, min(g0 + MG, len(mch))))
                    ps = {}
                    for mi in grp:
                        m0, mm = mch[mi]
                        ps[mi] = pspool.tile([mm, BT], F32, name="ps_t", tag="ps")
                    ks = list(range(len(kch)))
                    if (g0 // MG) % 2 == 1:
                        ks = ks[::-1]
                    for pos, ki in enumerate(ks):
                        first, last = pos == 0, pos == len(ks) - 1
                        kk = kch[ki][1]
                        for mi in grp:
                            m0, mm = mch[mi]
                            for nb in range(0, BT, 512):
                                nc.tensor.matmul(
                                    ps[mi][:, nb:nb + 512],
                                    wt[ki][:kk, m0:m0 + mm],
                                    h_in[ki][:kk, nb:nb + 512],
                                    start=first, stop=last,
                                )
                    for mi in grp:
                        m0, mm = mch[mi]
                        amm = mm + 1 if (extra_row and mi == len(mch) - 1) else mm
                        ot = apool.tile([amm, BT], BF16, name="act_t", tag="a")
                        bias = bias_tiles.get(mi, 0.0)
                        bias = bias[:, :] if not isinstance(bias, float) else bias
                        nc.scalar.activation(
                            ot[:mm, :], ps[mi][:, :],
                            AF.Relu if relu else AF.Copy, bias=bias,
                        )
                        out.append(ot)
                return out

            import contextlib
            rep_ctx = tc.For_i(0, repeat, 1) if repeat > 1 else contextlib.nullcontext()
            with rep_ctx:
              for p in range(PASSES):
                b0 = p * BT
                # ---- load x.T slice (feature-major input)
                h = []
                for (k0, kk) in _chunks(DIMS[0]):
                    t = apool.tile([kk, BT], BF16, name="xt_t", tag="a")
                    nc.sync.dma_start(t[:, :], XT[k0:k0 + kk, b0:b0 + BT])
                    h.append(t)

                # ---- encoder
                h = feat_layer(h, eW[0], DIMS[0], DIMS[1], True, ebias["eb0"])
                h = feat_layer(h, eW[1], DIMS[1], DIMS[2], True, ebias["eb1"])
                h = feat_layer(h, eW[2], DIMS[2], DIMS[3], True, ebias["eb2"])
                z = feat_layer(h, eW[3], DIMS[3], LATENT, False, ebias["eb3"])

                # ---- SOM prep: zaug = [z0, z1, z2row, onesrow]
                zsq = []
                for zt in z:
                    s = apool.tile([128, BT], BF16, name="zsq_t", tag="a")
                    nc.vector.tensor_mul(s[:, :], zt[:, :], zt[:, :])
                    zsq.append(s)
                ztail = apool.tile([2, BT], BF16, name="ztail", tag="a")
                psz = pspool.tile([1, BT], F32, name="psz", tag="ps")
                for ki in range(2):
                    for nb in range(0, BT, 512):
                        nc.tensor.matmul(psz[:, nb:nb + 512], ones_col[:, :],
                                         zsq[ki][:, nb:nb + 512],
                                         start=(ki == 0), stop=(ki == 1))
                nc.scalar.activation(ztail[0:1, :], psz[:, :], AF.Copy)
                nc.sync.dma_start(ztail[1:2, :], ONESR[:, b0:b0 + BT])
                zaug = z + [ztail]

                # ---- SOM distances (batch-major out)
                zch = ZCH
                for bt_i in range(BT // 128):
                    bsl = slice(bt_i * 128, (bt_i + 1) * 128)
                    dtile = opool.tile([128, P_PROTO], F32, name="dtile", tag="o")
                    # ki outer / half inner so both 512-halves share one
                    # stationary load (deduped by _dedupe_ldweights)
                    psd = [pspool.tile([128, 512], F32, name="psd", tag="ps")
                           for _ in range(2)]
                    for ki in range(len(zch)):
                        for half in range(2):
                            nsl = slice(half * 512, (half + 1) * 512)
                            nc.tensor.matmul(
                                psd[half][:, :], zaug[ki][:, bsl],
                                proto_tiles[ki][:, nsl],
                                start=(ki == 0), stop=(ki == len(zch) - 1),
                            )
                    for half in range(2):
                        nsl = slice(half * 512, (half + 1) * 512)
                        nc.scalar.activation(dtile[:, nsl], psd[half][:, :], AF.Relu)
                    nc.sync.dma_start(DOUT[b0 + bt_i * 128: b0 + (bt_i + 1) * 128, :],
                                      dtile[:, :])

                # ---- decoder
                h = feat_layer(z, dW[0], LATENT, DIMS[3], True, ebias["db3"])
                h = feat_layer(h, dW[1], DIMS[3], DIMS[2], True, ebias["db2"])
                h = feat_layer(h, dW[2], DIMS[2], DIMS[1], True, ebias["db1"],
                               extra_row=True)
                # fused output-bias row (matches dW0aug row 2000)
                nc.sync.dma_start(h[-1][80:81, :], ONESR[:, b0:b0 + BT])

                # ---- decoder output layer (batch-major out, bias folded)
                kch = _chunks(DIMS[1])
                kch = kch[:-1] + [(kch[-1][0], kch[-1][1] + 1)]  # 80 -> 81 rows
                w0t = []
                for (k0, kk) in kch:
                    t = wpool.tile([kk, DIMS[0]], BF16, name="w0_t", tag="w")
                    nc.sync.dma_start(t[:, :], DW0AUG[k0:k0 + kk, :])
                    w0t.append(t)
                NSPLIT = ((0, 512), (512, DIMS[0] - 512))
                for bt_i in range(BT // 128):
                    bsl = slice(bt_i * 128, (bt_i + 1) * 128)
                    dec = opool.tile([128, DIMS[0]], F32, name="dec_t", tag="o")
                    psd2 = [pspool.tile([128, nn], F32, name="psd2", tag="ps")
                            for (n0, nn) in NSPLIT]
                    for ki in range(len(kch)):
                        for hi, (n0, nn) in enumerate(NSPLIT):
                            nc.tensor.matmul(
                                psd2[hi][:, :], h[ki][:, bsl], w0t[ki][:, n0:n0 + nn],
                                start=(ki == 0), stop=(ki == len(kch) - 1),
                            )
                    for hi, (n0, nn) in enumerate(NSPLIT):
                        nc.scalar.activation(dec[:, n0:n0 + nn], psd2[hi][:, :],
                                             AF.Copy)
                    nc.sync.dma_start(DEC[b0 + bt_i * 128: b0 + (bt_i + 1) * 128, :],
                                      dec[:, :])

    if os.environ.get("DESOM_NO_DEDUP", "0") != "1":
        _dedupe_ldweights(nc)
    _split_multiwaits(nc)
    return nc


_PROGRAM_CACHE = {}


def _get_program(has_bias, repeat=1):
    key = (has_bias, repeat, os.environ.get("DESOM_NO_DEDUP", "0"))
    if key not in _PROGRAM_CACHE:
        _PROGRAM_CACHE[key] = _build_program(has_bias, repeat)
    return _PROGRAM_CACHE[key]


def _prepare_in_maps(inputs):
    bf = ml_dtypes.bfloat16
    f32 = np.float32

    def as_np(v, dt=f32):
        return np.asarray(v, dtype=dt)

    x = as_np(inputs["x"])
    P = as_np(inputs["prototypes"])
    dW0 = as_np(inputs["dW0"])
    db0 = as_np(inputs["db0"])

    xt = np.ascontiguousarray(x.astype(bf).T)           # [784, B_TOTAL]
    p2 = (P.astype(np.float64) ** 2).sum(1).astype(f32)
    proto_aug = np.concatenate(
        [(-2.0 * P.T), np.ones((1, P_PROTO), f32), p2[None, :]], axis=0
    ).astype(bf)                                        # [258, 1024]
    dW0aug = np.concatenate([dW0, db0[None, :]], axis=0).astype(bf)  # [2001, 784]

    weights = {f"eW{i}": as_np(inputs[f"eW{i}"]).astype(bf) for i in range(4)}
    weights.update({f"dW{j}": as_np(inputs[f"dW{j}"]).astype(bf) for j in (3, 2, 1)})

    biases = {nm: as_np(inputs[nm]) for nm in
              ("eb0", "eb1", "eb2", "eb3", "db3", "db2", "db1")}
    has_bias = any(np.any(b != 0) for b in biases.values())

    in_maps = []
    for c in range(N_CORES):
        m = {
            "xt": np.ascontiguousarray(xt[:, c * B_SHARD:(c + 1) * B_SHARD]),
            "dW0aug": dW0aug,
            "protoT_aug": proto_aug,
            "ones_r": np.ones((1, B_SHARD), bf),
        }
        m.update(weights)
        if has_bias:
            m.update(biases)
        in_maps.append(m)
    return in_maps, has_bias


def _run(inputs, trace=False):
    in_maps, has_bias = _prepare_in_maps(inputs)
    nc = _get_program(has_bias)
    res = run_bass_kernel_spmd(nc, in_maps, core_ids=list(range(N_CORES)),
                               trace=trace)
    decoded = np.concatenate([res.results[c]["decoded"] for c in range(N_CORES)], axis=0)
    dist = np.concatenate([res.results[c]["dist"] for c in range(N_CORES)], axis=0)
    return (decoded, dist), res


def kernel(**inputs):
    import time as _time
    last = None
    for attempt in range(3):
        try:
            (decoded, dist), _ = _run(inputs, trace=False)
            return decoded, dist
        except Exception as e:  # transient NRT/dispatch failure: retry
            last = e
            _time.sleep(5 * (attempt + 1))
    raise last


# ---------------------------------------------------------------------------
# Timing path: cached jit executable + device-resident inputs, so repeated
# calls measure dispatch+execute without the ~250 MB axon input transfer.
# ---------------------------------------------------------------------------
_EXEC_CACHE = {}


def _get_executable(nc, chain=1):
    if (id(nc), chain) in _EXEC_CACHE:
        return _EXEC_CACHE[(id(nc), chain)]
    import jax
    from concourse.bass2jax import (
        _bass_exec_p, install_neuronx_cc_hook, partition_id_tensor)
    from jax.experimental.shard_map import shard_map
    from jax.sharding import Mesh, PartitionSpec

    install_neuronx_cc_hook()
    part_name = nc.partition_id_tensor.name if nc.partition_id_tensor else None
    in_names, out_names, out_avals = [], [], []
    for alloc in nc.m.functions[0].allocations:
        if not isinstance(alloc, mybir.MemoryLocationSet):
            continue
        name = alloc.memorylocations[0].name
        if alloc.kind == "ExternalInput":
            if name != part_name:
                in_names.append(name)
        elif alloc.kind == "ExternalOutput":
            out_names.append(name)
            out_avals.append(jax.core.ShapedArray(
                tuple(alloc.tensor_shape), mybir.dt.np(alloc.dtype)))
    n_params = len(in_names)
    all_names = list(in_names) + list(out_names)
    if part_name is not None:
        all_names.append(part_name)
    all_names = tuple(all_names)

    n_outs = len(out_names)

    def _body(*args):
        ins = list(args[:n_params])
        state = list(args[n_params:n_params + n_outs])
        pid = [partition_id_tensor()] if part_name is not None else []
        for _ in range(chain):
            state = list(_bass_exec_p.bind(
                *ins, *state, *pid, out_avals=tuple(out_avals),
                in_names=all_names, out_names=tuple(out_names),
                lowering_input_output_aliases=(),
                sim_require_finite=True, sim_require_nnan=True, nc=nc))
        return tuple(state)

    devices = jax.devices()[:N_CORES]
    mesh = Mesh(np.asarray(devices), ("core",))
    nio = n_params + len(out_names)
    fn = jax.jit(
        shard_map(_body, mesh=mesh,
                  in_specs=(PartitionSpec("core"),) * nio,
                  out_specs=(PartitionSpec("core"),) * len(out_names),
                  check_rep=False),
        keep_unused=True,
    )
    _EXEC_CACHE[(id(nc), chain)] = (fn, in_names, out_names, out_avals, mesh)
    return _EXEC_CACHE[(id(nc), chain)]


def _run_on_device(nc, in_maps, iters):
    """Dispatch nc via a cached jit executable with device-resident inputs;
    returns (per-core results, list of wall times)."""
    import time as _time
    import jax
    from jax.sharding import NamedSharding, PartitionSpec

    fn, in_names, out_names, out_avals, mesh = _get_executable(nc)
    sh = NamedSharding(mesh, PartitionSpec("core"))
    concat_in = [np.concatenate([m[name] for m in in_maps], axis=0)
                 for name in in_names]
    zeros = [np.zeros((N_CORES * a.shape[0], *a.shape[1:]), a.dtype)
             for a in out_avals]
    dev_args = [jax.device_put(a, sh) for a in concat_in + zeros]
    outs = fn(*dev_args)
    jax.block_until_ready(outs)
    times = []
    for _ in range(iters):
        t0 = _time.perf_counter()
        jax.block_until_ready(fn(*dev_args))
        times.append(_time.perf_counter() - t0)
    res = {}
    for i, name in enumerate(out_names):
        arr = np.asarray(outs[i]).reshape(N_CORES, *out_avals[i].shape)
        res[name] = np.concatenate(list(arr), axis=0)
    return res, times


def _timed_run(inputs, iters=6, repeat=32):
    """Returns outputs plus an estimated per-execution time.  The kernel body
    is repeated `repeat` times inside one NEFF via a hardware loop so the
    ~50-110 ms remote-dispatch overhead amortizes away:
        per_exec = (T(repeat) - T(1)) / (repeat - 1)."""
    in_maps, has_bias = _prepare_in_maps(inputs)
    res1, t1s = _run_on_device(_get_program(has_bias, 1), in_maps, iters)
    _, tNs = _run_on_device(_get_program(has_bias, repeat), in_maps, iters)
    per_exec = (min(tNs) - min(t1s)) / (repeat - 1)
    return (res1["decoded"], res1["dist"]), {
        "per_exec_s": per_exec, "t1_ms": [t * 1e3 for t in t1s],
        "tN_ms": [t * 1e3 for t in tNs], "chain": repeat,
    }
